# revision 2
# baseline (speedup 1.0000x reference)
"""LocalAutoCorr2D Trainium2 kernel.

out[b,c,i,j,dy,dx] = sum_{y,x valid} x[b,c,4i+y,4j+x] * x[b,c,4i+y+sy,4j+x+sx]
with (sy,sx) = (dy-4, dx-4), windows 8x8 at stride 4 on a 96x96 image,
zero-padded at window boundaries.

Strategy (per core, batch-sharded over 8 cores):
  - out[s] == out[-s] (autocorr symmetry) -> only 40 canonical shift classes.
  - For each canonical shift (sy>=0, sx): product Q = x .* shift(x) on the
    Vector engine (fp16, 2x mode), with h on partitions so the vertical
    box-sum can run on the Tensor engine as a 0/1-weight matmul; the
    horizontal box-sum is folded into PSUM accumulation across <=8 matmuls
    whose rhs APs are column-shifted strided views of Q.
  - Vertical shifts are pre-materialized as partition-shifted SBUF copies
    (DMA); odd horizontal shifts get +1-column-shifted copies so every
    product op keeps 4B alignment for the DVE 2x perf mode.
"""

import functools
import os
import sys

import numpy as np

sys.path.insert(0, "/opt/trn_rl_repo")

import concourse.bass as bass  # noqa: E402
import concourse.bacc as bacc  # noqa: E402
import concourse.mybir as mybir  # noqa: E402
from concourse import bass_utils  # noqa: E402
from concourse.tile import TileContext  # noqa: E402

B, C, H, W = 8, 64, 96, 96
KH = KW = 8
SH = SW = 4
NH = NW = 23
NCORES = 8
CW = C * W  # 6144 flat (c,w) columns
PAD = 4  # column padding so sx in [-4,4] offsets stay in-tile

fp32 = mybir.dt.float32
fp16 = mybir.dt.float16


def _canonical_cells():
    """Map canonical shift (sy>=0, sx) -> list of output cells (dy,dx)."""
    cells = {}
    for dy in range(8):
        for dx in range(8):
            sy, sx = dy - 4, dx - 4
            key = (sy, sx) if (sy > 0 or (sy == 0 and sx >= 0)) else (-sy, -sx)
            cells.setdefault(key, []).append((dy, dx))
    assert len(cells) == 40
    return cells


def _amat_np():
    """Vertical box-sum matrices, stacked: A[h, sy*23+i] = 1 if 0<=h-4i<8-sy."""
    a = np.zeros((H, 5 * NH), np.float16)
    for sy in range(5):
        for i in range(NH):
            a[4 * i : 4 * i + 8 - sy, sy * NH + i] = 1.0
    return a


C_CHUNKS = [(0, 22), (22, 43), (43, 64)]  # N = 506/483/483 <= 512 per matmul


def build_nc():
    nc = bacc.Bacc()
    x_dram = nc.dram_tensor("x", [C, H, W], fp32, kind="ExternalInput")
    amat_dram = nc.dram_tensor("amat", [H, 5 * NH], fp16, kind="ExternalInput")
    out_dram = nc.dram_tensor("out", [8, 8, NH, CW_OUT := C * NW], fp32,
                              kind="ExternalOutput")

    cells = _canonical_cells()
    # order: by sy so early shifts only need T00, copies land meanwhile
    order = sorted(cells.keys(), key=lambda s: (s[0], abs(s[1])))

    with TileContext(nc) as tc:
        with (
            tc.tile_pool(name="const", bufs=1) as cpool,
            tc.tile_pool(name="xstage", bufs=1) as xpool,
            tc.tile_pool(name="tcop", bufs=1) as tpool,
            tc.tile_pool(name="q", bufs=2) as qpool,
            tc.tile_pool(name="o", bufs=2) as opool,
            tc.tile_pool(name="ps", bufs=2, space="PSUM") as ppool,
        ):
            amat_t = cpool.tile([H, 5 * NH], fp16)
            nc.gpsimd.dma_start(amat_t, amat_dram[:, :])

            xr = x_dram[:, :, :].rearrange("c h w -> h c w")

            # T tiles: T[(sy,p)][r, PAD+k] = x16[r+sy, k+p]
            tt = {}
            for sy in range(5):
                for p in range(2):
                    if (sy, p) == (0, 0):
                        t = tpool.tile([H, PAD + CW + PAD], fp16, name="T00")
                    else:
                        t = tpool.tile([H, PAD + CW + PAD], fp16,
                                       name=f"T{sy}{p}")
                    tt[(sy, p)] = t
                    nc.vector.memset(t[:, 0:PAD], 0.0)
                    nc.vector.memset(t[:, PAD + CW : PAD + CW + PAD], 0.0)

            t00 = tt[(0, 0)]
            # load fp32 in 2 chunks, then DVE-convert to fp16 in 2 halves so
            # every downstream consumer of t00 has a single (same-engine)
            # producer -- avoids blowing the per-instruction sem-wait limit.
            x32 = xpool.tile([H, CW], fp32)
            for k in range(2):
                sl = slice(k * 3072, (k + 1) * 3072)
                nc.gpsimd.dma_start(x32[:, sl], xr[:, k * 32 : (k + 1) * 32, :])
                nc.vector.tensor_copy(
                    t00[:, PAD + k * 3072 : PAD + (k + 1) * 3072], x32[:, sl]
                )
            # shifted copies via SBUF->SBUF DMA
            for (sy, p), t in tt.items():
                if (sy, p) == (0, 0):
                    continue
                nc.gpsimd.dma_start(
                    t[0 : H - sy, PAD : PAD + CW - p],
                    t00[sy:H, PAD + p : PAD + CW],
                )

            for (sy, sx) in order:
                p = sx & 1
                hv = H - sy
                q = qpool.tile([H, CW], fp16, tag="q")
                off = PAD + sx - p
                nc.vector.tensor_mul(
                    q[0:hv, :],
                    t00[0:hv, PAD : PAD + CW],
                    tt[(sy, p)][0:hv, off : off + CW],
                )
                qv = q.rearrange("h (c w) -> h c w", c=C)
                a_k = amat_t[0:hv, sy * NH : (sy + 1) * NH]
                xlist = list(range(max(0, -sx), 8 - max(0, sx)))
                o_t = opool.tile([NH, C * NW], fp32, tag="o")
                for ci, (c0, c1) in enumerate(C_CHUNKS):
                    pt = ppool.tile([NH, (c1 - c0) * NW], fp32, tag=f"ps{ci}")
                    for xi, xx in enumerate(xlist):
                        rhs = qv[0:hv, c0:c1, xx : xx + 4 * NW - 3 : 4]
                        nc.tensor.matmul(
                            pt, a_k, rhs,
                            start=(xi == 0), stop=(xi == len(xlist) - 1),
                        )
                    nc.scalar.copy(o_t[:, c0 * NW : c1 * NW], pt)
                for (dy, dx) in cells[(sy, sx)]:
                    nc.gpsimd.dma_start(out_dram[dy, dx], o_t)

    if not nc.is_finalized():
        nc.finalize()
    return nc


@functools.lru_cache(maxsize=1)
def _get_nc():
    return build_nc()


def _in_maps(x):
    amat = _amat_np()
    return [
        {"x": np.ascontiguousarray(x[b]), "amat": amat} for b in range(NCORES)
    ]


def kernel(**inputs) -> np.ndarray:
    x = np.asarray(inputs["x"], dtype=np.float32)
    assert x.shape == (B, C, H, W)
    nc = _get_nc()
    in_maps = _in_maps(x)
    res = bass_utils.run_bass_kernel_spmd(
        nc, in_maps, core_ids=list(range(NCORES)),
        trace=bool(int(os.environ.get("KERNEL_TRACE", "0"))),
    )
    outs = np.stack([r["out"] for r in res.results])  # [B, dy, dx, i, (c j)]
    outs = outs.reshape(B, 8, 8, NH, C, NW)
    # -> [B, c, i, j, dy, dx]
    full = outs.transpose(0, 4, 3, 5, 1, 2)
    return np.ascontiguousarray(full).astype(np.float32)


if __name__ == "__main__":
    rng = np.random.default_rng(0)
    x = rng.standard_normal((B, C, H, W), dtype=np.float32)
    y = kernel(x=x)
    print("out", y.shape, y.dtype, float(np.abs(y).max()))



# revision 4
# speedup vs baseline: 1.2545x; 1.2545x over previous
"""LocalAutoCorr2D Trainium2 kernel.

out[b,c,i,j,dy,dx] = sum_{y,x valid} x[b,c,4i+y,4j+x] * x[b,c,4i+y+sy,4j+x+sx]
with (sy,sx) = (dy-4, dx-4), windows 8x8 at stride 4 on a 96x96 image,
zero-padded at window boundaries.

Strategy (per core, batch-sharded over 8 cores):
  - out[s] == out[-s] (autocorr symmetry) -> only 40 canonical shift classes.
  - x is host-prepped into a PHASE-MAJOR fp16 layout [h, (r, c, j)] with
    w = 4j + r, so that the matmul rhs views (fixed r, a 23-wide j window
    per c) are contiguous: the PE streams at full rate (a strided stride-4
    rhs runs at half rate).
  - Per shift: product Q = x .* shift(x) on the Vector engine (fp16 2x
    mode, flat contiguous views; a +1-element-shifted host copy keeps all
    operands 4B-aligned). Vertical box-sum via 0/1-weight matmul (h on
    partitions), horizontal box-sum folded into PSUM accumulation across
    <=8 matmuls whose rhs are (r, j-window) views of Q.
  - Scalar engine evacuates PSUM -> SBUF; GpSimd queues the output DMAs.
"""

import functools
import os
import sys

import numpy as np

sys.path.insert(0, "/opt/trn_rl_repo")

import concourse.bass as bass  # noqa: E402
import concourse.bacc as bacc  # noqa: E402
import concourse.mybir as mybir  # noqa: E402
from concourse import bass_utils  # noqa: E402
from concourse.tile import TileContext  # noqa: E402

B, C, H, W = 8, 64, 96, 96
KH = KW = 8
SH = SW = 4
NH = NW = 23
NCORES = 8

JP = 24           # j' positions per r-block (w = 4j + r)
BLK = C * JP      # 1536 elements per r-block
FLAT = 4 * BLK    # 6144
NV = 5            # vertical shift copies v=0..4 stacked in the free dim
BASE = 8          # leading pad elements (AP validity for negative offsets)
TAIL = 16
XCOLS = BASE + NV * FLAT + TAIL

fp32 = mybir.dt.float32
fp16 = mybir.dt.float16


def _canonical_cells():
    """Map canonical shift (sy>=0, sx) -> list of output cells (dy,dx)."""
    cells = {}
    for dy in range(8):
        for dx in range(8):
            sy, sx = dy - 4, dx - 4
            key = (sy, sx) if (sy > 0 or (sy == 0 and sx >= 0)) else (-sy, -sx)
            cells.setdefault(key, []).append((dy, dx))
    assert len(cells) == 40
    return cells


def _amat_np():
    """Vertical box-sum matrices, stacked: A[h, sy*23+i] = 1 if 0<=h-4i<8-sy."""
    a = np.zeros((H, 5 * NH), np.float16)
    for sy in range(5):
        for i in range(NH):
            a[4 * i : 4 * i + 8 - sy, sy * NH + i] = 1.0
    return a


def _prep_x(xb):
    """[C,H,W] fp32 -> (xa, xo) phase-major fp16 [H, XCOLS].

    xa[h, BASE + v*FLAT + (r,c,j)] = x[h+v, c, 4j+r]  (0 beyond the image);
    xo[k] = xa[k+1] (the +1-element copy keeps odd offsets 4B-aligned)."""
    t = xb.transpose(1, 2, 0)  # [h, w, c]
    pm = t.reshape(H, JP, 4, C).transpose(0, 2, 3, 1)  # [h, r, c, j]
    flat = np.ascontiguousarray(pm.reshape(H, FLAT)).astype(np.float16)
    stack = np.zeros((H, NV * FLAT), np.float16)
    for v in range(NV):
        stack[0 : H - v, v * FLAT : (v + 1) * FLAT] = flat[v:H]
    xa = np.zeros((H, XCOLS), np.float16)
    xa[:, BASE : BASE + NV * FLAT] = stack
    xo = np.zeros((H, XCOLS), np.float16)
    xo[:, BASE - 1 : BASE - 1 + NV * FLAT] = stack
    return xa, xo


C_CHUNKS = [(0, 22), (22, 43), (43, 64)]  # N = 506/483/483 <= 512 per matmul


def _order(cells):
    """sy=0 shifts first (their stack block lands first), then by growing
    |sx| so the PE builds backlog early and drains it on the cheap
    small-Lx shifts at the end."""
    return sorted(cells.keys(), key=lambda s: (s[0], abs(s[1])))


def build_nc():
    nc = bacc.Bacc()
    xa_dram = nc.dram_tensor("xa", [H, XCOLS], fp16, kind="ExternalInput")
    xo_dram = nc.dram_tensor("xo", [H, XCOLS], fp16, kind="ExternalInput")
    amat_dram = nc.dram_tensor("amat", [H, 5 * NH], fp16, kind="ExternalInput")
    out_dram = nc.dram_tensor("out", [8, 8, NH, C * NW], fp32,
                              kind="ExternalOutput")

    cells = _canonical_cells()
    order = _order(cells)

    with TileContext(nc) as tc:
        with (
            tc.tile_pool(name="const", bufs=1) as cpool,
            tc.tile_pool(name="q", bufs=3) as qpool,
            tc.tile_pool(name="o", bufs=3) as opool,
            tc.tile_pool(name="ps", bufs=2, space="PSUM") as ppool,
        ):
            amat_t = cpool.tile([H, 5 * NH], fp16)
            nc.gpsimd.dma_start(amat_t, amat_dram[:, :])
            xa_t = cpool.tile([H, XCOLS], fp16)
            xo_t = cpool.tile([H, XCOLS], fp16)
            # chunked so the v=0 block (first consumer) lands first
            bounds = [0] + [BASE + v * FLAT for v in range(1, NV)] + [XCOLS]
            for lo, hi in zip(bounds[:-1], bounds[1:]):
                nc.gpsimd.dma_start(xa_t[:, lo:hi], xa_dram[:, lo:hi])
                nc.gpsimd.dma_start(xo_t[:, lo:hi], xo_dram[:, lo:hi])

            for (sy, sx) in order:
                s = sx % 4          # python %: s in [0,4) also for sx<0
                a = (sx - s) // 4
                hv = H - sy
                q = qpool.tile([H, FLAT], fp16, tag="q")

                def mul(flo, fhi, delta):
                    # q[h, f] = x[h, f] * x[h+sy, f+delta-sy*FLAT] on
                    # f in [flo, fhi); the sy shift is baked into the stack.
                    src, off = (xa_t, BASE + delta) if delta % 2 == 0 else \
                               (xo_t, BASE + delta - 1)
                    nc.vector.tensor_mul(
                        q[0:hv, flo:fhi],
                        xa_t[0:hv, BASE + flo : BASE + fhi],
                        src[0:hv, off + flo : off + fhi],
                    )

                lenA = (4 - s) * BLK
                mul(0, lenA, sy * FLAT + s * BLK + a)
                if s:
                    mul(lenA, FLAT, sy * FLAT + (s - 4) * BLK + a + 1)

                qv = q.rearrange("h (r c j) -> h r c j", r=4, c=C)
                a_k = amat_t[0:hv, sy * NH : (sy + 1) * NH]
                xlist = list(range(max(0, -sx), 8 - max(0, sx)))
                o_t = opool.tile([NH, C * NW], fp32, tag="o")
                for ci, (c0, c1) in enumerate(C_CHUNKS):
                    pt = ppool.tile([NH, (c1 - c0) * NW], fp32, tag=f"ps{ci}")
                    for xi, xx in enumerate(xlist):
                        rhs = qv[0:hv, xx % 4, c0:c1, xx // 4 : xx // 4 + NW]
                        nc.tensor.matmul(
                            pt, a_k, rhs,
                            start=(xi == 0), stop=(xi == len(xlist) - 1),
                        )
                    nc.scalar.copy(o_t[:, c0 * NW : c1 * NW], pt)
                for (dy, dx) in cells[(sy, sx)]:
                    nc.gpsimd.dma_start(out_dram[dy, dx], o_t)

    if not nc.is_finalized():
        nc.finalize()
    return nc


@functools.lru_cache(maxsize=1)
def _get_nc():
    return build_nc()


def _in_maps(x):
    amat = _amat_np()
    maps = []
    for b in range(NCORES):
        xa, xo = _prep_x(x[b])
        maps.append({"xa": xa, "xo": xo, "amat": amat})
    return maps


def kernel(**inputs) -> np.ndarray:
    x = np.asarray(inputs["x"], dtype=np.float32)
    assert x.shape == (B, C, H, W)
    nc = _get_nc()
    in_maps = _in_maps(x)
    res = bass_utils.run_bass_kernel_spmd(
        nc, in_maps, core_ids=list(range(NCORES)),
        trace=bool(int(os.environ.get("KERNEL_TRACE", "0"))),
    )
    outs = np.stack([r["out"] for r in res.results])  # [B, dy, dx, i, (c j)]
    outs = outs.reshape(B, 8, 8, NH, C, NW)
    # -> [B, c, i, j, dy, dx]
    full = outs.transpose(0, 4, 3, 5, 1, 2)
    return np.ascontiguousarray(full).astype(np.float32)


if __name__ == "__main__":
    rng = np.random.default_rng(0)
    x = rng.standard_normal((B, C, H, W), dtype=np.float32)
    y = kernel(x=x)
    print("out", y.shape, y.dtype, float(np.abs(y).max()))


# revision 5
# speedup vs baseline: 2.2242x; 1.7729x over previous
"""LocalAutoCorr2D Trainium2 kernel.

out[b,c,i,j,dy,dx] = sum_{y,x valid} x[b,c,4i+y,4j+x] * x[b,c,4i+y+sy,4j+x+sx]
with (sy,sx) = (dy-4, dx-4), windows 8x8 at stride 4 on a 96x96 image,
zero-padded at window boundaries.

Strategy (per core, batch-sharded over 8 cores):
  - out[s] == out[-s] (autocorr symmetry) -> only 40 canonical shift classes.
  - x is host-prepped into a PHASE-MAJOR fp16 layout [h, (r, j, c)] with
    w = 4j + r and c innermost, so every matmul rhs view (fixed r, a
    23-j window, all c) is one FLAT contiguous slice: the PE streams at
    full rate (a strided or multi-dim rhs runs at ~half rate). The 5
    vertical shifts v=0..4 are also host-stacked along the free dim, so
    DVE products never need cross-partition operands.
  - Per shift: product Q = x .* shift(x) on the Vector engine (fp16 2x
    mode, flat contiguous views; all shift offsets are multiples of C=64
    elements, so alignment is automatic). Vertical box-sum via 0/1-weight
    matmul (h on partitions), horizontal box-sum folded into PSUM
    accumulation across <=8 matmuls over flat rhs slices of Q.
  - Scalar engine evacuates PSUM -> SBUF; GpSimd queues the output DMAs.
"""

import functools
import os
import sys

import numpy as np

sys.path.insert(0, "/opt/trn_rl_repo")

import concourse.bass as bass  # noqa: E402
import concourse.bacc as bacc  # noqa: E402
import concourse.mybir as mybir  # noqa: E402
from concourse import bass_utils  # noqa: E402
from concourse.tile import TileContext  # noqa: E402

B, C, H, W = 8, 64, 96, 96
KH = KW = 8
SH = SW = 4
NH = NW = 23
NCORES = 8

JP = 24           # j' positions per r-block (w = 4j + r)
BLK = C * JP      # 1536 elements per r-block
FLAT = 4 * BLK    # 6144
NV = 5            # vertical shift copies v=0..4 stacked in the free dim
BASE = 64         # leading pad elements (AP validity for negative offsets)
TAIL = 128
XCOLS = BASE + NV * FLAT + TAIL
N_CHUNKS = [(0, 512), (512, 1024), (1024, 1472)]  # flat cols per PSUM bank

fp32 = mybir.dt.float32
fp16 = mybir.dt.float16


def _canonical_cells():
    """Map canonical shift (sy>=0, sx) -> list of output cells (dy,dx)."""
    cells = {}
    for dy in range(8):
        for dx in range(8):
            sy, sx = dy - 4, dx - 4
            key = (sy, sx) if (sy > 0 or (sy == 0 and sx >= 0)) else (-sy, -sx)
            cells.setdefault(key, []).append((dy, dx))
    assert len(cells) == 40
    return cells


def _amat_np():
    """Vertical box-sum matrices, stacked: A[h, sy*23+i] = 1 if 0<=h-4i<8-sy."""
    a = np.zeros((H, 5 * NH), np.float16)
    for sy in range(5):
        for i in range(NH):
            a[4 * i : 4 * i + 8 - sy, sy * NH + i] = 1.0
    return a


def _prep_x(xb):
    """[C,H,W] fp32 -> xa phase-major fp16 [H, XCOLS].

    xa[h, BASE + v*FLAT + (r,j,c)] = x[h+v, c, 4j+r]  (0 beyond the image)."""
    t = xb.transpose(1, 2, 0)  # [h, w, c]
    pm = t.reshape(H, JP, 4, C).transpose(0, 2, 1, 3)  # [h, r, j, c]
    flat = np.ascontiguousarray(pm.reshape(H, FLAT)).astype(np.float16)
    xa = np.zeros((H, XCOLS), np.float16)
    for v in range(NV):
        xa[0 : H - v, BASE + v * FLAT : BASE + (v + 1) * FLAT] = flat[v:H]
    return xa


def _order(cells):
    """sy=0 shifts first (their stack block lands first), then by growing
    |sx| so the PE builds backlog early and drains it on the cheap
    small-Lx shifts at the end."""
    return sorted(cells.keys(), key=lambda s: (s[0], abs(s[1])))


def build_nc():
    nc = bacc.Bacc()
    xa_dram = nc.dram_tensor("xa", [H, XCOLS], fp16, kind="ExternalInput")
    amat_dram = nc.dram_tensor("amat", [H, 5 * NH], fp16, kind="ExternalInput")
    out_dram = nc.dram_tensor("out", [8, 8, NH, NW * C], fp32,
                              kind="ExternalOutput")

    cells = _canonical_cells()
    order = _order(cells)

    with TileContext(nc) as tc:
        with (
            tc.tile_pool(name="const", bufs=1) as cpool,
            tc.tile_pool(name="q", bufs=3) as qpool,
            tc.tile_pool(name="o", bufs=3) as opool,
            tc.tile_pool(name="ps", bufs=2, space="PSUM") as ppool,
        ):
            amat_t = cpool.tile([H, 5 * NH], fp16)
            nc.gpsimd.dma_start(amat_t, amat_dram[:, :])
            xa_t = cpool.tile([H, XCOLS], fp16)
            # chunked so the v=0 block (first consumer) lands first
            bounds = [0] + [BASE + v * FLAT for v in range(1, NV)] + [XCOLS]
            for lo, hi in zip(bounds[:-1], bounds[1:]):
                nc.gpsimd.dma_start(xa_t[:, lo:hi], xa_dram[:, lo:hi])

            for (sy, sx) in order:
                s = sx % 4          # python %: s in [0,4) also for sx<0
                a = (sx - s) // 4
                hv = H - sy
                q = qpool.tile([H, FLAT], fp16, tag="q")

                def mul(flo, fhi, delta):
                    # q[h, f] = x[h, f] * x[h+sy, f+delta-sy*FLAT] on
                    # f in [flo, fhi); the sy shift is baked into the stack.
                    off = BASE + delta
                    nc.vector.tensor_mul(
                        q[0:hv, flo:fhi],
                        xa_t[0:hv, BASE + flo : BASE + fhi],
                        xa_t[0:hv, off + flo : off + fhi],
                    )

                lenA = (4 - s) * BLK
                mul(0, lenA, sy * FLAT + s * BLK + C * a)
                if s:
                    mul(lenA, FLAT, sy * FLAT + (s - 4) * BLK + C * (a + 1))

                a_k = amat_t[0:hv, sy * NH : (sy + 1) * NH]
                xlist = list(range(max(0, -sx), 8 - max(0, sx)))
                o_t = opool.tile([NH, NW * C], fp32, tag="o")
                for ci, (n0, n1) in enumerate(N_CHUNKS):
                    pt = ppool.tile([NH, n1 - n0], fp32, tag=f"ps{ci}")
                    for xi, xx in enumerate(xlist):
                        base = (xx % 4) * BLK + C * (xx // 4)
                        rhs = q[0:hv, base + n0 : base + n1]
                        nc.tensor.matmul(
                            pt, a_k, rhs,
                            start=(xi == 0), stop=(xi == len(xlist) - 1),
                        )
                    nc.scalar.copy(o_t[:, n0:n1], pt)
                for (dy, dx) in cells[(sy, sx)]:
                    nc.gpsimd.dma_start(out_dram[dy, dx], o_t)

    if not nc.is_finalized():
        nc.finalize()
    return nc


@functools.lru_cache(maxsize=1)
def _get_nc():
    return build_nc()


def _in_maps(x):
    amat = _amat_np()
    return [{"xa": _prep_x(x[b]), "amat": amat} for b in range(NCORES)]


def kernel(**inputs) -> np.ndarray:
    x = np.asarray(inputs["x"], dtype=np.float32)
    assert x.shape == (B, C, H, W)
    nc = _get_nc()
    in_maps = _in_maps(x)
    res = bass_utils.run_bass_kernel_spmd(
        nc, in_maps, core_ids=list(range(NCORES)),
        trace=bool(int(os.environ.get("KERNEL_TRACE", "0"))),
    )
    outs = np.stack([r["out"] for r in res.results])  # [B, dy, dx, i, (j c)]
    outs = outs.reshape(B, 8, 8, NH, NW, C)
    # -> [B, c, i, j, dy, dx]
    full = outs.transpose(0, 5, 3, 4, 1, 2)
    return np.ascontiguousarray(full).astype(np.float32)


if __name__ == "__main__":
    rng = np.random.default_rng(0)
    x = rng.standard_normal((B, C, H, W), dtype=np.float32)
    y = kernel(x=x)
    print("out", y.shape, y.dtype, float(np.abs(y).max()))


# revision 6
# speedup vs baseline: 2.3476x; 1.0555x over previous
"""LocalAutoCorr2D Trainium2 kernel.

out[b,c,i,j,dy,dx] = sum_{y,x valid} x[b,c,4i+y,4j+x] * x[b,c,4i+y+sy,4j+x+sx]
with (sy,sx) = (dy-4, dx-4), windows 8x8 at stride 4 on a 96x96 image,
zero-padded at window boundaries.

Strategy (per core, batch-sharded over 8 cores):
  - out[s] == out[-s] (autocorr symmetry) -> only 40 canonical shift classes.
  - x is host-prepped into a PHASE-MAJOR fp16 layout [h, (r, j, c)] with
    w = 4j + r and c innermost, so every matmul rhs view (fixed r, a
    23-j window, all c) is one FLAT contiguous slice: the PE streams at
    full rate (a strided or multi-dim rhs runs at ~half rate). The 5
    vertical shifts v=0..4 are also host-stacked along the free dim, so
    DVE products never need cross-partition operands.
  - Per shift: product Q = x .* shift(x) on the Vector engine (fp16 2x
    mode, flat contiguous views; all shift offsets are multiples of C=64
    elements, so alignment is automatic). Vertical box-sum via 0/1-weight
    matmul (h on partitions), horizontal box-sum folded into PSUM
    accumulation across <=8 matmuls over flat rhs slices of Q.
  - Scalar engine evacuates PSUM -> SBUF; GpSimd queues the output DMAs.
"""

import functools
import os
import sys

import numpy as np

sys.path.insert(0, "/opt/trn_rl_repo")

import concourse.bass as bass  # noqa: E402
import concourse.bacc as bacc  # noqa: E402
import concourse.mybir as mybir  # noqa: E402
from concourse import bass_utils  # noqa: E402
from concourse.tile import TileContext  # noqa: E402

B, C, H, W = 8, 64, 96, 96
KH = KW = 8
SH = SW = 4
NH = NW = 23
NCORES = 8

JP = 24           # j' positions per r-block (w = 4j + r)
BLK = C * JP      # 1536 elements per r-block
FLAT = 4 * BLK    # 6144
NV = 5            # vertical shift copies v=0..4 stacked in the free dim
BASE = 64         # leading pad elements (AP validity for negative offsets)
TAIL = 128
XCOLS = BASE + NV * FLAT + TAIL
N_CHUNKS = [(0, 512), (512, 1024), (1024, 1472)]  # flat cols per PSUM bank

fp32 = mybir.dt.float32
fp16 = mybir.dt.float16


def _canonical_cells():
    """Map canonical shift (sy>=0, sx) -> list of output cells (dy,dx)."""
    cells = {}
    for dy in range(8):
        for dx in range(8):
            sy, sx = dy - 4, dx - 4
            key = (sy, sx) if (sy > 0 or (sy == 0 and sx >= 0)) else (-sy, -sx)
            cells.setdefault(key, []).append((dy, dx))
    assert len(cells) == 40
    return cells


def _amat_np():
    """Vertical box-sum matrices, stacked: A[h, sy*23+i] = 1 if 0<=h-4i<8-sy."""
    a = np.zeros((H, 5 * NH), np.float16)
    for sy in range(5):
        for i in range(NH):
            a[4 * i : 4 * i + 8 - sy, sy * NH + i] = 1.0
    return a


def _prep_x(xb):
    """[C,H,W] fp32 -> xa phase-major fp16 [H, XCOLS].

    xa[h, BASE + v*FLAT + (r,j,c)] = x[h+v, c, 4j+r]  (0 beyond the image)."""
    t = xb.transpose(1, 2, 0)  # [h, w, c]
    pm = t.reshape(H, JP, 4, C).transpose(0, 2, 1, 3)  # [h, r, j, c]
    flat = np.ascontiguousarray(pm.reshape(H, FLAT)).astype(np.float16)
    xa = np.zeros((H, XCOLS), np.float16)
    for v in range(NV):
        xa[0 : H - v, BASE + v * FLAT : BASE + (v + 1) * FLAT] = flat[v:H]
    return xa


def _order(cells):
    """sy=0 shifts first (their stack block lands first), then by growing
    |sx| so the PE builds backlog early; (4,0) moved to the very end so
    the PE drains on a big-Lx shift instead of starving."""
    o = sorted(cells.keys(), key=lambda s: (s[0], abs(s[1])))
    o.remove((4, 0))
    o.append((4, 0))
    return o


def build_nc():
    nc = bacc.Bacc()
    xa_dram = nc.dram_tensor("xa", [H, XCOLS], fp16, kind="ExternalInput")
    amat_dram = nc.dram_tensor("amat", [H, 5 * NH], fp16, kind="ExternalInput")
    out_dram = nc.dram_tensor("out", [8, 8, NH, NW * C], fp16,
                              kind="ExternalOutput")

    cells = _canonical_cells()
    order = _order(cells)

    with TileContext(nc) as tc:
        with (
            tc.tile_pool(name="const", bufs=1) as cpool,
            tc.tile_pool(name="q", bufs=4) as qpool,
            tc.tile_pool(name="o", bufs=3) as opool,
            tc.tile_pool(name="ps", bufs=2, space="PSUM") as ppool,
        ):
            amat_t = cpool.tile([H, 5 * NH], fp16)
            nc.gpsimd.dma_start(amat_t, amat_dram[:, :])
            xa_t = cpool.tile([H, XCOLS], fp16)
            # chunked so the v=0 block (first consumer) lands first;
            # v=0 in halves so the (0,0) square can start on the first half
            bounds = [0, BASE + FLAT // 2] + \
                [BASE + v * FLAT for v in range(1, NV)] + [XCOLS]
            for lo, hi in zip(bounds[:-1], bounds[1:]):
                nc.gpsimd.dma_start(xa_t[:, lo:hi], xa_dram[:, lo:hi])

            for (sy, sx) in order:
                s = sx % 4          # python %: s in [0,4) also for sx<0
                a = (sx - s) // 4
                hv = H - sy
                q = qpool.tile([H, FLAT], fp16, tag="q")

                def mul(flo, fhi, delta):
                    # q[h, f] = x[h, f] * x[h+sy, f+delta-sy*FLAT] on
                    # f in [flo, fhi); the sy shift is baked into the stack.
                    off = BASE + delta
                    nc.vector.tensor_mul(
                        q[0:hv, flo:fhi],
                        xa_t[0:hv, BASE + flo : BASE + fhi],
                        xa_t[0:hv, off + flo : off + fhi],
                    )

                if (sy, sx) == (0, 0):
                    # x^2 on the Act engine: frees the DVE and starts as
                    # soon as each half of the v=0 DMA chunk lands
                    for lo, hi in [(0, FLAT // 2), (FLAT // 2, FLAT)]:
                        nc.scalar.activation(
                            q[:, lo:hi], xa_t[:, BASE + lo : BASE + hi],
                            mybir.ActivationFunctionType.Square,
                        )
                else:
                    lenA = (4 - s) * BLK
                    mul(0, lenA, sy * FLAT + s * BLK + C * a)
                    if s:
                        mul(lenA, FLAT,
                            sy * FLAT + (s - 4) * BLK + C * (a + 1))

                a_k = amat_t[0:hv, sy * NH : (sy + 1) * NH]
                xlist = list(range(max(0, -sx), 8 - max(0, sx)))
                o_t = opool.tile([NH, NW * C], fp16, tag="o")
                for ci, (n0, n1) in enumerate(N_CHUNKS):
                    pt = ppool.tile([NH, n1 - n0], fp32, tag=f"ps{ci}")
                    for xi, xx in enumerate(xlist):
                        base = (xx % 4) * BLK + C * (xx // 4)
                        rhs = q[0:hv, base + n0 : base + n1]
                        nc.tensor.matmul(
                            pt, a_k, rhs,
                            start=(xi == 0), stop=(xi == len(xlist) - 1),
                        )
                    nc.scalar.copy(o_t[:, n0:n1], pt)
                for (dy, dx) in cells[(sy, sx)]:
                    nc.gpsimd.dma_start(out_dram[dy, dx], o_t)

    if not nc.is_finalized():
        nc.finalize()
    return nc


@functools.lru_cache(maxsize=1)
def _get_nc():
    return build_nc()


def _in_maps(x):
    amat = _amat_np()
    return [{"xa": _prep_x(x[b]), "amat": amat} for b in range(NCORES)]


def kernel(**inputs) -> np.ndarray:
    x = np.asarray(inputs["x"], dtype=np.float32)
    assert x.shape == (B, C, H, W)
    nc = _get_nc()
    in_maps = _in_maps(x)
    res = bass_utils.run_bass_kernel_spmd(
        nc, in_maps, core_ids=list(range(NCORES)),
        trace=bool(int(os.environ.get("KERNEL_TRACE", "0"))),
    )
    outs = np.stack([r["out"] for r in res.results])  # [B, dy, dx, i, (j c)]
    outs = outs.reshape(B, 8, 8, NH, NW, C)
    # -> [B, c, i, j, dy, dx]
    full = outs.transpose(0, 5, 3, 4, 1, 2)
    return np.ascontiguousarray(full).astype(np.float32)


if __name__ == "__main__":
    rng = np.random.default_rng(0)
    x = rng.standard_normal((B, C, H, W), dtype=np.float32)
    y = kernel(x=x)
    print("out", y.shape, y.dtype, float(np.abs(y).max()))
